# revision 10
# baseline (speedup 1.0000x reference)
"""BiMultiHeadAttention (GLIP-style bidirectional cross-attention) on 8 TRN2 cores.

Sharding: 8 shards = 2 batches x 4 vision-token chunks (T=16000 -> 4000/core).
Each core computes, for its (batch, t-slice):
  - q/val_v projections of its v rows; k/val_l projections of l (replicated)
  - vision-direction attention (softmax over language axis s) end-to-end,
    including the out_v projection -> its slice of out_v
  - language-direction partial sums (numerator over its t rows + partial
    denominator), which the host merges across the 4 cores of each batch and
    projects with W_ol (tiny: 0.4% of total FLOPs).

Layouts (per core, one batch):
  vT  [256, 4000]  (vision dim on partitions)        host-transposed input
  qT  [128, 2, 4000]  e=(4 heads x 32d) per tile     from projection
  kT  [128, 2, 256]   same head packing
  val_v_aug [128, 32(tc), 8(h), 33]  rows=t%125, col 32 = 1.0 (denominator)
  val_l_aug [128, 2(sc), 8(h), 33]   rows=s%128, scaled by mask, col 32 = mask
Attention is computed twice (both orientations) because the two softmax
directions need opposite operands on the contraction partitions:
  pass V: PT[s=128, t=500] per (h, s-chunk) -> exp -> E_v -> AV [t=125,33]
          -> normalize by Z (col 32) -> transpose -> out_v proj -> DMA out
  pass L: P[t=125, s=256] x4 heads -> exp -> out_l partial [33, s=256]
          accumulated over all 32 t-chunks in PSUM, flushed at the end.
Biases are exact: b_v/b_l folded on-device (per-partition adds); b_vv/b_vl/
b_ov/b_ol folded on host (attention rows sum to 1 after normalization, so
value biases commute with the attention average).
"""

import numpy as np

import concourse.bass as bass
import concourse.mybir as mybir
import concourse.tile as tile
from concourse import bacc
from concourse.bass_utils import run_bass_kernel_spmd
from concourse.masks import make_identity
from concourse.tile_rust import add_dep_helper

F32 = mybir.dt.float32
F32R = mybir.dt.float32r

# Problem constants (hardcoded per harness contract)
B, T, S = 2, 16000, 256
V_DIM, L_DIM, EMBED, HEADS = 256, 768, 256, 8
HEAD_DIM = EMBED // HEADS          # 32
SCALE = HEAD_DIM ** (-0.5)
NT = T // 4                        # 4000 t-rows per core
TCH = 125                          # t-chunk (partition grid)
NTC = NT // TCH                    # 32 chunks
TG = 500                           # t-group for pass-V QK (free dim)
NTG = NT // TG                     # 8 groups
SUBS = TG // TCH                   # 4 sub-chunks per group

# dtype config: use fp32r (4x faster PE, ~5e-4 rel err) on selected matmuls
CFG = {
    "qk_r": False,      # QK^T matmuls (both passes)
    "qproj_r": False,   # q/k projections
    "vproj_r": False,   # val_v / val_l projections
    "outl_r": False,    # out_l AV matmul
    "final_r": False,   # final out_v projection
}


def _r(ap, flag):
    return ap.bitcast(F32R) if flag else ap


def build_nc():
    nc = bacc.Bacc(None, target_bir_lowering=False)

    # DRAM I/O (per-core shard)
    vT = nc.dram_tensor("vT", [V_DIM, NT], F32, kind="ExternalInput")
    lT = nc.dram_tensor("lT", [L_DIM, S], F32, kind="ExternalInput")
    wqT = nc.dram_tensor("wqT", [V_DIM, EMBED], F32, kind="ExternalInput")
    wlT = nc.dram_tensor("wlT", [L_DIM, EMBED], F32, kind="ExternalInput")
    wvvT = nc.dram_tensor("wvvT", [V_DIM, EMBED], F32, kind="ExternalInput")
    wvlT = nc.dram_tensor("wvlT", [L_DIM, EMBED], F32, kind="ExternalInput")
    wovT = nc.dram_tensor("wovT", [EMBED, V_DIM], F32, kind="ExternalInput")
    biasq = nc.dram_tensor("biasq", [EMBED, 1], F32, kind="ExternalInput")
    biask = nc.dram_tensor("biask", [EMBED, 1], F32, kind="ExternalInput")
    maskf = nc.dram_tensor("maskf", [S, 1], F32, kind="ExternalInput")

    outv = nc.dram_tensor("outv", [NT, V_DIM], F32, kind="ExternalOutput")
    outl_part = nc.dram_tensor("outl_part", [HEADS, HEAD_DIM + 1, S], F32,
                               kind="ExternalOutput")

    with tile.TileContext(nc) as tc:
        with tc.tile_pool(name="persist", bufs=1) as persist:
            # Persistent SBUF tensors
            qT_t = persist.tile([128, 2, NT], F32, tag="qT")
            kT_t = persist.tile([128, 2, S], F32, tag="kT")
            vva_t = persist.tile([128, NTC, HEADS, HEAD_DIM + 1], F32, tag="vva")
            vla_t = persist.tile([128, 2, HEADS, HEAD_DIM + 1], F32, tag="vla")
            wov_t = persist.tile([128, 2, V_DIM], F32, tag="wov")
            mask_t = persist.tile([128, 2], F32, tag="mask")
            bq_t = persist.tile([128, 2], F32, tag="bq")
            bk_t = persist.tile([128, 2], F32, tag="bk")
            ident_t = persist.tile([TCH, TCH], F32, tag="ident")
            vT_t = persist.tile([128, 2, NT], F32, tag="vT")

            nc.sync.dma_start(out=wov_t[:, 0, :], in_=wovT[0:128, :])
            nc.sync.dma_start(out=wov_t[:, 1, :], in_=wovT[128:256, :])
            nc.sync.dma_start(out=mask_t[:, 0:1], in_=maskf[0:128, :])
            nc.sync.dma_start(out=mask_t[:, 1:2], in_=maskf[128:256, :])
            nc.sync.dma_start(out=bq_t[:, 0:1], in_=biasq[0:128, :])
            nc.sync.dma_start(out=bq_t[:, 1:2], in_=biasq[128:256, :])
            nc.sync.dma_start(out=bk_t[:, 0:1], in_=biask[0:128, :])
            nc.sync.dma_start(out=bk_t[:, 1:2], in_=biask[128:256, :])
            nc.sync.dma_start(out=vT_t[:, 0, :], in_=vT[0:128, :])
            nc.sync.dma_start(out=vT_t[:, 1, :], in_=vT[128:256, :])
            make_identity(nc, ident_t)
            # ones column of val_v_aug (partition rows beyond 125 harmless)
            nc.vector.memset(vva_t[:, :, :, HEAD_DIM:HEAD_DIM + 1], 1.0)

            # ---------------- stage 0: projections ----------------
            with tc.tile_pool(name="s0", bufs=1) as s0, \
                 tc.tile_pool(name="s0ps", bufs=4, space="PSUM") as s0ps:
                lT_t = s0.tile([128, 6, S], F32, tag="lT")
                wl_t = s0.tile([128, 6, EMBED], F32, tag="wl")
                wvl_t = s0.tile([128, 6, EMBED], F32, tag="wvl")
                wq_t = s0.tile([128, 2, EMBED], F32, tag="wq")
                wvv_t = s0.tile([128, 2, EMBED], F32, tag="wvv")
                for kc in range(6):
                    nc.sync.dma_start(out=lT_t[:, kc, :], in_=lT[kc * 128:(kc + 1) * 128, :])
                    nc.sync.dma_start(out=wl_t[:, kc, :], in_=wlT[kc * 128:(kc + 1) * 128, :])
                    nc.sync.dma_start(out=wvl_t[:, kc, :], in_=wvlT[kc * 128:(kc + 1) * 128, :])
                for kc in range(2):
                    nc.sync.dma_start(out=wq_t[:, kc, :], in_=wqT[kc * 128:(kc + 1) * 128, :])
                    nc.sync.dma_start(out=wvv_t[:, kc, :], in_=wvvT[kc * 128:(kc + 1) * 128, :])

                # kT[e, s] = W_l @ l^T   (+ b_l per-partition)
                for ec in range(2):
                    pk = s0ps.tile([128, S], F32, tag="ps0")
                    for kc in range(6):
                        nc.tensor.matmul(
                            pk, _r(wl_t[:, kc, ec * 128:(ec + 1) * 128], CFG["qproj_r"]),
                            _r(lT_t[:, kc, :], CFG["qproj_r"]),
                            start=(kc == 0), stop=(kc == 5))
                    nc.vector.tensor_scalar_add(kT_t[:, ec, :], pk, bk_t[:, ec:ec + 1])

                # val_l[s, e] = l @ W_vl^T, scaled by mask, into aug layout
                for sc in range(2):
                    pv = s0ps.tile([128, EMBED], F32, tag="ps0")
                    for kc in range(6):
                        nc.tensor.matmul(
                            pv, _r(lT_t[:, kc, sc * 128:(sc + 1) * 128], CFG["vproj_r"]),
                            _r(wvl_t[:, kc, :], CFG["vproj_r"]),
                            start=(kc == 0), stop=(kc == 5))
                    nc.vector.tensor_scalar_mul(
                        vla_t[:, sc, :, 0:HEAD_DIM],
                        pv.rearrange("p (h d) -> p h d", h=HEADS),
                        mask_t[:, sc:sc + 1])
                    for h in range(HEADS):
                        nc.vector.tensor_copy(
                            vla_t[:, sc, h, HEAD_DIM:HEAD_DIM + 1],
                            mask_t[:, sc:sc + 1])

                # qT[e, t] = (SCALE*W_v) @ v^T (+ SCALE*b_v)
                for ec in range(2):
                    for tg in range(NTG):
                        pq = s0ps.tile([128, TG], F32, tag="ps0")
                        for kc in range(2):
                            nc.tensor.matmul(
                                pq, _r(wq_t[:, kc, ec * 128:(ec + 1) * 128], CFG["qproj_r"]),
                                _r(vT_t[:, kc, tg * TG:(tg + 1) * TG], CFG["qproj_r"]),
                                start=(kc == 0), stop=(kc == 1))
                        nc.vector.tensor_scalar_add(
                            qT_t[:, ec, tg * TG:(tg + 1) * TG], pq, bq_t[:, ec:ec + 1])

                # val_v[t, e] = v @ W_vv^T into aug layout (ones col preset)
                for tci in range(NTC):
                    pvv = s0ps.tile([TCH, EMBED], F32, tag="ps0")
                    for kc in range(2):
                        nc.tensor.matmul(
                            pvv, _r(vT_t[:, kc, tci * TCH:(tci + 1) * TCH], CFG["vproj_r"]),
                            _r(wvv_t[:, kc, :], CFG["vproj_r"]),
                            start=(kc == 0), stop=(kc == 1))
                    nc.vector.tensor_copy(
                        vva_t[:TCH, tci, :, 0:HEAD_DIM],
                        pvv.rearrange("p (h d) -> p h d", h=HEADS))

            # ---------------- loop 1: vision-direction (pass V) ----------------
            with tc.tile_pool(name="ptps", bufs=3, space="PSUM") as ptps, \
                 tc.tile_pool(name="avps", bufs=2, space="PSUM") as avps, \
                 tc.tile_pool(name="fps", bufs=2, space="PSUM") as fps, \
                 tc.tile_pool(name="ev", bufs=20) as evp, \
                 tc.tile_pool(name="l1sb", bufs=4) as l1sb:
                for tg in range(NTG):
                    ev_tiles = {}
                    for h in range(HEADS):
                        hh, hr = h // 4, 32 * (h % 4)
                        for sc in range(2):
                            pt = ptps.tile([128, TG], F32, tag="pt")
                            nc.tensor.matmul(
                                pt,
                                _r(kT_t[hr:hr + 32, hh, sc * 128:(sc + 1) * 128], CFG["qk_r"]),
                                _r(qT_t[hr:hr + 32, hh, tg * TG:(tg + 1) * TG], CFG["qk_r"]),
                                tile_position=(hr, 0))
                            ev = evp.tile([128, TG], F32, tag="ev")
                            nc.scalar.activation(ev, pt, mybir.ActivationFunctionType.Exp)
                            ev_tiles[(h, sc)] = ev

                    for sub in range(SUBS):
                        tci = tg * SUBS + sub
                        pav = avps.tile([TCH, HEADS, HEAD_DIM + 1], F32, tag="pav")
                        # all 8 head-regions share one PSUM bank = one 2KB
                        # zero region: start only on the first matmul, stop
                        # only on the last, and chain them so the scheduler
                        # keeps that order on PE.
                        prev_mm = None
                        for h in range(HEADS):
                            for sc in range(2):
                                mm = nc.tensor.matmul(
                                    pav[:, h, :],
                                    ev_tiles[(h, sc)][:, sub * TCH:(sub + 1) * TCH],
                                    vla_t[:, sc, h, :],
                                    start=(h == 0 and sc == 0),
                                    stop=(h == HEADS - 1 and sc == 1))
                                if prev_mm is not None:
                                    add_dep_helper(
                                        mm.ins, prev_mm.ins, sync=False,
                                        reason="psum zero-region order")
                                prev_mm = mm
                        # normalize: rows 0:32 of each head / Z (col 32)
                        zrec = l1sb.tile([TCH, HEADS], F32, tag="zrec")
                        nc.vector.reciprocal(zrec, pav[:, :, HEAD_DIM:HEAD_DIM + 1].rearrange("p h one -> p (h one)"))
                        ovn = l1sb.tile([TCH, EMBED], F32, tag="ovn")
                        for h in range(HEADS):
                            nc.vector.tensor_scalar_mul(
                                ovn[:, h * HEAD_DIM:(h + 1) * HEAD_DIM],
                                pav[:, h, 0:HEAD_DIM],
                                zrec[:, h:h + 1])
                        # transpose to [e, t] and project
                        pf = fps.tile([TCH, V_DIM], F32, tag="pf")
                        for ec in range(2):
                            ptr = fps.tile([128, TCH], F32, tag="ptr", bufs=1)
                            nc.tensor.transpose(ptr, ovn[:, ec * 128:(ec + 1) * 128], ident_t)
                            ovT = l1sb.tile([128, TCH], F32, tag="ovT")
                            nc.vector.tensor_copy(ovT, ptr)
                            nc.tensor.matmul(
                                pf, _r(ovT, CFG["final_r"]),
                                _r(wov_t[:, ec, :], CFG["final_r"]),
                                start=(ec == 0), stop=(ec == 1))
                        fo = l1sb.tile([TCH, V_DIM], F32, tag="fo")
                        nc.vector.tensor_copy(fo, pf)
                        nc.sync.dma_start(out=outv[tci * TCH:(tci + 1) * TCH, :], in_=fo)

            # ---------------- loop 2: language-direction (pass L) ----------------
            with tc.tile_pool(name="plps", bufs=1, space="PSUM") as plps, \
                 tc.tile_pool(name="olps", bufs=4, space="PSUM") as olps, \
                 tc.tile_pool(name="l2sb", bufs=3) as l2sb:
                for hq in range(2):
                    po = [olps.tile([HEAD_DIM + 1, S], F32, tag="po",
                                    name=f"po_{hq}_{j}") for j in range(4)]
                    for tci in range(NTC):
                        # one full PSUM bank per row-group head: concurrent
                        # row-group drains must never share a bank
                        pl = plps.tile([TCH, 4, 512], F32, tag="pl")
                        for j in range(4):
                            h = hq * 4 + j
                            hh, hr = h // 4, 32 * (h % 4)
                            nc.tensor.matmul(
                                pl[:, j, 0:S],
                                _r(qT_t[hr:hr + 32, hh, tci * TCH:(tci + 1) * TCH], CFG["qk_r"]),
                                _r(kT_t[hr:hr + 32, hh, :], CFG["qk_r"]),
                                tile_position=(hr, 0))
                        el = l2sb.tile([TCH, 4, S], F32, tag="el")
                        nc.scalar.activation(el, pl[:, :, 0:S],
                                             mybir.ActivationFunctionType.Exp)
                        for j in range(4):
                            h = hq * 4 + j
                            nc.tensor.matmul(
                                po[j],
                                _r(vva_t[:TCH, tci, h, :], CFG["outl_r"]),
                                _r(el[:, j, :], CFG["outl_r"]),
                                start=(tci == 0), stop=(tci == NTC - 1))
                    for j in range(4):
                        h = hq * 4 + j
                        ol = l2sb.tile([HEAD_DIM + 1, S], F32, tag="ol")
                        nc.vector.tensor_copy(ol, po[j])
                        nc.sync.dma_start(out=outl_part[h, :, :], in_=ol)

    nc.finalize()
    return nc


_NC_CACHE = None


def _get_nc():
    global _NC_CACHE
    if _NC_CACHE is None:
        _NC_CACHE = build_nc()
    return _NC_CACHE


def kernel(v, l, attention_mask_l, W_v, b_v, W_l, b_l, W_vv, b_vv,
           W_vl, b_vl, W_ov, b_ov, W_ol, b_ol):
    v = np.asarray(v, dtype=np.float32)
    l = np.asarray(l, dtype=np.float32)
    mask = np.asarray(attention_mask_l)
    W_v = np.asarray(W_v, dtype=np.float32); b_v = np.asarray(b_v, dtype=np.float32)
    W_l = np.asarray(W_l, dtype=np.float32); b_l = np.asarray(b_l, dtype=np.float32)
    W_vv = np.asarray(W_vv, dtype=np.float32); b_vv = np.asarray(b_vv, dtype=np.float32)
    W_vl = np.asarray(W_vl, dtype=np.float32); b_vl = np.asarray(b_vl, dtype=np.float32)
    W_ov = np.asarray(W_ov, dtype=np.float32); b_ov = np.asarray(b_ov, dtype=np.float32)
    W_ol = np.asarray(W_ol, dtype=np.float32); b_ol = np.asarray(b_ol, dtype=np.float32)

    shared = {
        "wqT": np.ascontiguousarray((W_v * SCALE).T),
        "wlT": np.ascontiguousarray(W_l.T),
        "wvvT": np.ascontiguousarray(W_vv.T),
        "wvlT": np.ascontiguousarray(W_vl.T),
        "wovT": np.ascontiguousarray(W_ov.T),
        "biasq": np.ascontiguousarray((b_v * SCALE)[:, None]),
        "biask": np.ascontiguousarray(b_l[:, None]),
    }
    lT_b = [np.ascontiguousarray(l[b].T) for b in range(B)]
    maskf_b = [np.ascontiguousarray((mask[b] != 0).astype(np.float32)[:, None])
               for b in range(B)]
    in_maps = []
    for c in range(8):
        b, tcn = c // 4, c % 4
        in_maps.append({
            "vT": np.ascontiguousarray(v[b, tcn * NT:(tcn + 1) * NT].T),
            "lT": lT_b[b],
            "maskf": maskf_b[b],
            **shared,
        })

    nc = _get_nc()
    res = run_bass_kernel_spmd(nc, in_maps, core_ids=list(range(8)))

    # ---- host gather / merge ----
    out_v = np.empty((B, T, V_DIM), dtype=np.float32)
    bias_ov_eff = b_vl @ W_ov.T + b_ov          # exact: attn_v rows sum to 1
    for c in range(8):
        b, tcn = c // 4, c % 4
        out_v[b, tcn * NT:(tcn + 1) * NT] = res.results[c]["outv"] + bias_ov_eff

    bias_ol_eff = b_vv @ W_ol.T + b_ol
    out_l = np.empty((B, S, L_DIM), dtype=np.float32)
    for b in range(B):
        acc = np.zeros((HEADS, HEAD_DIM + 1, S), dtype=np.float64)
        for tcn in range(4):
            acc += res.results[b * 4 + tcn]["outl_part"]
        num = acc[:, :HEAD_DIM, :]              # [h, d, s]
        z = acc[:, HEAD_DIM, :]                 # [h, s]
        attn_out = (num / z[:, None, :]).transpose(2, 0, 1).reshape(S, EMBED)
        out_l[b] = (attn_out @ W_ol.T).astype(np.float32) + bias_ol_eff

    return out_v, out_l
